# revision 45
# baseline (speedup 1.0000x reference)
"""Adaptive top-k MoE router on 8 TRN2 NeuronCores.

Data-parallel over tokens: each core routes T/8 = 2048 tokens.
Per core: weight-stationary bf16 matmul with PE column tiling — the two
64-column halves of the PE array compute two different 512-token tiles
against the same stationary weights, so each PSUM half holds complete
fp32 logits (single accumulation chain -> single bf16 rounding, matching
the reference einsum bit-for-bit). Then: bf16 cast -> PE transpose to
token-major -> ACT exp/ln + DVE softmax/entropy/top-8 -> adaptive-k mask
+ renormalize -> int32/bf16 outputs.

Host side: shards hidden along T, pre-packed so every device DMA is a
dense 1MB transfer with 8KB contiguous per partition; replicates a
rearranged weight; reassembles the full outputs.
"""

import numpy as np
import ml_dtypes
from contextlib import ExitStack

P = 128            # SBUF partitions
E = 64             # experts
H = 4096           # hidden dim
T_FULL = 16384     # total tokens
N_CORES = 8
T_LOC = T_FULL // N_CORES   # 2048 tokens per core
NCH = H // P                # 32 contraction chunks
TT = 512                    # token tile (matmul free dim)
NT = T_LOC // TT            # 4 token tiles
G = TT // P                 # 4 groups of 128 tokens per tile
NG = T_LOC // P             # 16 groups per core
QJ = 4                      # chunks per DMA quad
NQ = NCH // QJ              # 8 quads
SUP = 2 * TT                # supertile: two token tiles sharing one matmul
NSUP = T_LOC // SUP         # 2 supertiles

BF16 = ml_dtypes.bfloat16
IDB = np.eye(64, dtype=ml_dtypes.bfloat16)

_CACHE = {}


def _pin_act_tables():
    """Make every ACT table set except natural_log_exp_and_others look like
    it contains none of {Copy, Exp, Ln}, so the table chooser emits a single
    ACT_TABLE_LOAD instead of thrashing between the exp and ln sets.
    Dict order/length is preserved (index == act_func_set_id)."""
    import concourse.hw_specs as hw_specs
    import concourse.bacc as bacc_mod
    import concourse.mybir as mybir

    if _CACHE.get("act_patched"):
        return
    orig = hw_specs.get_activation_tables
    mine = {mybir.ActivationFunctionType.Copy,
            mybir.ActivationFunctionType.Exp,
            mybir.ActivationFunctionType.Ln}

    def patched(module_arch):
        tables = dict(orig(module_arch))
        out = {}
        for name, funcs in tables.items():
            if name == "natural_log_exp_and_others":
                out[name] = funcs
            else:
                out[name] = funcs - mine
        return out

    hw_specs.get_activation_tables = patched
    bacc_mod.get_activation_tables = patched
    _CACHE["act_patched"] = True


def build_nc(t_loc=T_LOC):
    """Build the single-core Bass graph (same NEFF runs SPMD on all 8 cores)."""
    import concourse.bass as bass
    import concourse.bacc as bacc
    import concourse.mybir as mybir
    from concourse import tile
    from concourse.tile_rust import add_dep_helper

    _pin_act_tables()

    nsup = t_loc // SUP

    f32 = mybir.dt.float32
    bf16 = mybir.dt.bfloat16
    i32 = mybir.dt.int32
    u32 = mybir.dt.uint32
    AX = mybir.AxisListType.X
    OP = mybir.AluOpType
    AF = mybir.ActivationFunctionType

    nc = bacc.Bacc("TRN2", target_bir_lowering=False, debug=False)

    # ht[s, q, p, j*SUP + u] = hiddenT[(4q+j)*128 + p, s*SUP + u]
    ht = nc.declare_dram_parameter("ht", [nsup, NQ, P, QJ * SUP], bf16,
                                   isOutput=False)
    wt = nc.declare_dram_parameter("wt", [P, NCH * E], bf16, isOutput=False)
    idb = nc.declare_dram_parameter("idb", [E, E], bf16, isOutput=False)
    oi = nc.declare_dram_parameter("oi", [P, NG, 4], i32, isOutput=True)
    ow = nc.declare_dram_parameter("ow", [P, NG, 4], bf16, isOutput=True)
    ok = nc.declare_dram_parameter("ok", [P, NG], i32, isOutput=True)

    with tile.TileContext(nc) as tc, ExitStack() as ctx:
        const = ctx.enter_context(tc.tile_pool(name="const", bufs=1))
        htp = ctx.enter_context(tc.tile_pool(name="htp", bufs=5))
        mmps = ctx.enter_context(
            tc.tile_pool(name="mmps", bufs=2, space=bass.MemorySpace.PSUM))
        trps = ctx.enter_context(
            tc.tile_pool(name="trps", bufs=2, space=bass.MemorySpace.PSUM))
        work = ctx.enter_context(tc.tile_pool(name="work", bufs=2))
        outp = ctx.enter_context(tc.tile_pool(name="outp", bufs=1))

        wt_s = const.tile([P, NCH * E], bf16)
        nc.scalar.dma_start(wt_s[:], wt.ap())
        identb = const.tile([E, E], bf16)
        nc.scalar.dma_start(identb[:], idb.ap())
        oi_s = outp.tile([P, NG, 4], i32)
        ow_s = outp.tile([P, NG, 4], bf16)
        ok_s = outp.tile([P, NG], i32)

        def emit_mm(s):
            # Column-tiled matmuls: PE columns 0-63 compute the first
            # 512-token tile, columns 64-127 the second, against the same
            # stationary weight chunk. Each PSUM half accumulates its
            # tokens' complete logitsT over all 32 chunks.
            lg_ps = mmps.tile([P, TT], f32, tag="lgps")
            mid_mm = None
            for q in range(NQ):
                htile = htp.tile([P, QJ, SUP], bf16, tag="ht")
                nc.sync.dma_start(htile[:], ht.ap()[s, q])
                for j in range(QJ):
                    c = QJ * q + j
                    for half in range(2):
                        # skip_group_check: the sim's PSUM-group tracker is
                        # not partition-base aware; the halves are disjoint
                        # partition ranges of the bank.
                        mm = nc.tensor.matmul(
                            lg_ps[half * E:(half + 1) * E, :],
                            wt_s[:, bass.ts(c, E)],
                            htile[:, j, bass.ts(half, TT)],
                            start=(c == 0), stop=(c == NCH - 1),
                            tile_position=(0, half * E),
                            skip_group_check=True)
                    if q == NQ // 2 and j == 0:
                        mid_mm = mm
            return lg_ps, mid_mm

        def emit_post(s, lg_ps, anchor):
            # single fp32 -> bf16 rounding (the reference's einsum output),
            # split lo/hi so each consumer starts after a half-size copy
            sb = work.tile([P, TT], bf16, tag="sb")
            nc.scalar.copy(sb[E:P, :], lg_ps[E:P, :])
            # move the second half to partition base 0 (HWDGE SBUF copy)
            # so every PE transpose runs in the proven base-0 form
            sb_hi = work.tile([E, TT], bf16, tag="sbhi")
            nc.scalar.dma_start(sb_hi[:], sb[E:P, :])
            nc.scalar.copy(sb[0:E, :], lg_ps[0:E, :])

            for half in range(2):
                t = 2 * s + half
                src = sb[0:E, :] if half == 0 else sb_hi[:]

                # ---- PE transpose to token-major: (128 tok, G, 64 exp)
                tr_ps = trps.tile([P, G, E], bf16, tag="trps")
                for g in range(G):
                    tr = nc.tensor.transpose(
                        tr_ps[:, g, :], src[:, bass.ts(g, P)], identb[:])
                    if anchor is not None:
                        # scheduling-only: keep transposes after the NEXT
                        # supertile's mid-matmul so the PE never head-of-line
                        # stalls on this merge chain
                        add_dep_helper(tr.ins, anchor.ins, sync=False,
                                       reason="tr after next supertile mms")
                lgt = work.tile([P, G, E], bf16, tag="lgt")
                nc.vector.tensor_copy(lgt[:], tr_ps[:])

                # ---- softmax (fp32): y = x - max (single-rounded fp32,
                # bit-identical to ACT's fused bias), one batched exp
                mneg = work.tile([P, G, 1], f32, tag="mneg")
                nc.vector.reduce_max(mneg[:, :, 0], lgt[:], axis=AX, negate=True)
                y = work.tile([P, G, E], f32, tag="y")
                ya, yb = bass.broadcast_tensor_aps(lgt[:], mneg[:])
                nc.vector.tensor_tensor(y[:], ya, yb, OP.add)
                pe_t = work.tile([P, G, E], f32, tag="pe")
                nc.scalar.activation(pe_t[:], y[:], AF.Exp)
                # top-8 first: independent of the entropy chain, packs the
                # DVE FIFO while ACT runs
                mv = work.tile([P, G, 8], f32, tag="mv")
                mi = work.tile([P, G, 8], u32, tag="mi")
                for g in range(G):
                    nc.vector.max(mv[:, g, :], pe_t[:, g, :])
                    nc.vector.max_index(mi[:, g, :], mv[:, g, :], pe_t[:, g, :])
                zs = work.tile([P, G, 1], f32, tag="zs")
                nc.vector.reduce_sum(zs[:, :, 0], pe_t[:], axis=AX)
                rz = work.tile([P, G, 1], f32, tag="rz")
                nc.vector.reciprocal(rz[:, :, 0], zs[:, :, 0])

                # ---- entropy_neg = sum p*ln(p) = rz*sum(pe*y) - ln(Z)
                # (drops the reference's +1e-9 epsilon: |delta| < 1e-7,
                #  three orders below the 3e-5 threshold margins)
                pl = work.tile([P, G, E], f32, tag="pl")
                nc.vector.tensor_tensor(pl[:], pe_t[:], y[:], OP.mult)
                s2 = work.tile([P, G, 1], f32, tag="s2")
                nc.vector.reduce_sum(s2[:, :, 0], pl[:], axis=AX)
                lnz = work.tile([P, G, 1], f32, tag="lnz")
                nc.scalar.activation(lnz[:, :, 0], zs[:, :, 0], AF.Ln)
                e1 = work.tile([P, G, 1], f32, tag="e1")
                nc.vector.tensor_tensor(e1[:], s2[:], rz[:], OP.mult)
                entn = work.tile([P, G], f32, tag="entn")
                nc.vector.tensor_tensor(entn[:], e1[:, :, 0], lnz[:, :, 0],
                                        OP.subtract)

                # ---- adaptive k: entropy<0.3 -> 1, >1.5 -> 4, else 2
                # entn = -entropy: k>=2 iff entn <= -0.3; k==4 iff entn < -1.5
                m2 = work.tile([P, G, 1], f32, tag="m2")
                nc.vector.tensor_scalar(m2[:, :, 0], entn[:], -0.3, None, OP.is_le)
                m4 = work.tile([P, G, 1], f32, tag="m4")
                nc.vector.tensor_scalar(m4[:, :, 0], entn[:], -1.5, None, OP.is_lt)
                kf = work.tile([P, G], f32, tag="kf")
                nc.vector.scalar_tensor_tensor(
                    kf[:], m4[:, :, 0], 2.0, m2[:, :, 0], OP.mult, OP.add)
                nc.vector.tensor_scalar_add(kf[:], kf[:], 1.0)
                nc.vector.tensor_copy(ok_s[:, bass.ts(t, G)], kf[:])

                # ---- active-slot mask (slot0 always, slot1 k>=2, 2/3 k==4)
                act = work.tile([P, G, 4], f32, tag="act")
                nc.vector.memset(act[:, :, 0:1], 1.0)
                nc.vector.tensor_copy(act[:, :, 1:2], m2[:])
                nc.vector.tensor_copy(act[:, :, 2:3], m4[:])
                nc.vector.tensor_copy(act[:, :, 3:4], m4[:])

                # ---- masked renormalized weights
                w4 = work.tile([P, G, 4], f32, tag="w4")
                nc.vector.tensor_tensor(w4[:], mv[:, :, 0:4], act[:], OP.mult)
                ws = work.tile([P, G], f32, tag="ws")
                nc.vector.reduce_sum(ws[:], w4[:], axis=AX)
                rw = work.tile([P, G, 1], f32, tag="rw")
                nc.vector.reciprocal(rw[:, :, 0], ws[:])
                wn = work.tile([P, G, 4], f32, tag="wn")
                wa_bc, wb_bc = bass.broadcast_tensor_aps(w4[:], rw[:])
                nc.vector.tensor_tensor(wn[:], wa_bc, wb_bc, OP.mult)
                nc.vector.tensor_copy(ow_s[:, bass.ts(t, G)], wn[:])

                # ---- indices: (idx+1)*active - 1 (inactive slots -> -1)
                idxf = work.tile([P, G, 4], f32, tag="idxf")
                nc.vector.tensor_copy(idxf[:], mi[:, :, 0:4])
                nc.vector.scalar_tensor_tensor(
                    idxf[:], idxf[:], 1.0, act[:], OP.add, OP.mult)
                nc.vector.tensor_scalar_add(idxf[:], idxf[:], -1.0)
                nc.vector.tensor_copy(oi_s[:, bass.ts(t, G)], idxf[:])
                # per-tile output stores: only the final tile's three small
                # DMAs sit between the last compute and the kernel drain
                gsl = bass.ts(t, G)
                nc.sync.dma_start(oi.ap()[:, gsl], oi_s[:, gsl])
                nc.sync.dma_start(ow.ap()[:, gsl], ow_s[:, gsl])
                nc.sync.dma_start(ok.ap()[:, gsl], ok_s[:, gsl])



        # software pipeline: supertile s streams while s-1 post-processes
        prev = None
        for s in range(nsup):
            lg, mid = emit_mm(s)
            if prev is not None:
                emit_post(s - 1, prev, mid)
            prev = lg
        emit_post(nsup - 1, prev, None)

    nc.finalize()
    return nc


def _get_nc():
    if "nc" not in _CACHE:
        _CACHE["nc"] = build_nc()
    return _CACHE["nc"]


def _prep_shards(hidden, weight):
    hidden = np.asarray(hidden)
    weight = np.asarray(weight)
    if hidden.dtype != BF16:
        hidden = hidden.astype(BF16)
    if weight.dtype != BF16:
        weight = weight.astype(BF16)
    # weight (E, H) -> wt[p, c*E + e] = weight[e, c*P + p]
    wt = np.ascontiguousarray(
        weight.reshape(E, NCH, P).transpose(2, 1, 0).reshape(P, NCH * E))
    # hidden (T, H) -> hiddenT (H, T), shard along tokens, pack per-DMA-dense:
    # ht[s, q, p, j*SUP + u] = hiddenT[(QJ*q+j)*P + p, s*SUP + u]
    ht_full = np.ascontiguousarray(hidden.T)
    in_maps = []
    for c in range(N_CORES):
        sh = ht_full[:, c * T_LOC:(c + 1) * T_LOC]          # (H, T_LOC)
        s5 = sh.reshape(NQ, QJ, P, NSUP, SUP)               # (q, j, p, s, u)
        ht_shard = np.ascontiguousarray(
            s5.transpose(3, 0, 2, 1, 4).reshape(NSUP, NQ, P, QJ * SUP))
        in_maps.append({"ht": ht_shard, "wt": wt, "idb": IDB})
    return in_maps


def _assemble(results):
    idx_parts, w_parts, k_parts = [], [], []
    for c in range(N_CORES):
        oi = np.asarray(results[c]["oi"])            # (P, NG, 4) int32
        ow = np.asarray(results[c]["ow"])            # (P, NG, 4) bf16
        ok = np.asarray(results[c]["ok"])            # (P, NG)   int32
        # token = g*128 + p  ->  [g, p, s]
        idx_parts.append(oi.transpose(1, 0, 2).reshape(T_LOC, 4))
        w_parts.append(ow.transpose(1, 0, 2).reshape(T_LOC, 4))
        k_parts.append(ok.transpose(1, 0).reshape(T_LOC))
    indices = np.concatenate(idx_parts, axis=0).astype(np.int32)
    weights = np.concatenate(w_parts, axis=0)
    if weights.dtype != BF16:
        weights = weights.view(BF16) if weights.dtype.itemsize == 2 \
            else weights.astype(BF16)
    k = np.concatenate(k_parts, axis=0).astype(np.int32)
    return indices, weights, k


def kernel(hidden, weight):
    from concourse.bass_utils import run_bass_kernel_spmd

    nc = _get_nc()
    in_maps = _prep_shards(hidden, weight)
    res = run_bass_kernel_spmd(nc, in_maps, core_ids=list(range(N_CORES)))
    return _assemble(res.results)


# revision 46
# speedup vs baseline: 1.0987x; 1.0987x over previous
"""Adaptive top-k MoE router on 8 TRN2 NeuronCores.

Data-parallel over tokens: each core routes T/8 = 2048 tokens.
Per core: weight-stationary bf16 matmul with PE column tiling — the two
64-column halves of the PE array compute two different 512-token tiles
against the same stationary weights, so each PSUM half holds complete
fp32 logits (single accumulation chain -> single bf16 rounding, matching
the reference einsum bit-for-bit). Then: bf16 cast -> PE transpose to
token-major -> ACT exp/ln + DVE softmax/entropy/top-8 -> adaptive-k mask
+ renormalize -> int32/bf16 outputs.

Host side: shards hidden along T, pre-packed so every device DMA is a
dense 1MB transfer with 8KB contiguous per partition; replicates a
rearranged weight; reassembles the full outputs.
"""

import numpy as np
import ml_dtypes
from contextlib import ExitStack

P = 128            # SBUF partitions
E = 64             # experts
H = 4096           # hidden dim
T_FULL = 16384     # total tokens
N_CORES = 8
T_LOC = T_FULL // N_CORES   # 2048 tokens per core
NCH = H // P                # 32 contraction chunks
TT = 512                    # token tile (matmul free dim)
NT = T_LOC // TT            # 4 token tiles
G = TT // P                 # 4 groups of 128 tokens per tile
NG = T_LOC // P             # 16 groups per core
QJ = 4                      # chunks per DMA quad
NQ = NCH // QJ              # 8 quads
SUP = 2 * TT                # supertile: two token tiles sharing one matmul
NSUP = T_LOC // SUP         # 2 supertiles

BF16 = ml_dtypes.bfloat16
IDB = np.eye(64, dtype=ml_dtypes.bfloat16)

_CACHE = {}


def _pin_act_tables():
    """Make every ACT table set except natural_log_exp_and_others look like
    it contains none of {Copy, Exp, Ln}, so the table chooser emits a single
    ACT_TABLE_LOAD instead of thrashing between the exp and ln sets.
    Dict order/length is preserved (index == act_func_set_id)."""
    import concourse.hw_specs as hw_specs
    import concourse.bacc as bacc_mod
    import concourse.mybir as mybir

    if _CACHE.get("act_patched"):
        return
    orig = hw_specs.get_activation_tables
    mine = {mybir.ActivationFunctionType.Copy,
            mybir.ActivationFunctionType.Exp,
            mybir.ActivationFunctionType.Ln}

    def patched(module_arch):
        tables = dict(orig(module_arch))
        out = {}
        for name, funcs in tables.items():
            if name == "natural_log_exp_and_others":
                out[name] = funcs
            else:
                out[name] = funcs - mine
        return out

    hw_specs.get_activation_tables = patched
    bacc_mod.get_activation_tables = patched
    _CACHE["act_patched"] = True


def build_nc(t_loc=T_LOC):
    """Build the single-core Bass graph (same NEFF runs SPMD on all 8 cores)."""
    import concourse.bass as bass
    import concourse.bacc as bacc
    import concourse.mybir as mybir
    from concourse import tile
    from concourse.tile_rust import add_dep_helper

    _pin_act_tables()

    nsup = t_loc // SUP

    f32 = mybir.dt.float32
    bf16 = mybir.dt.bfloat16
    i32 = mybir.dt.int32
    u32 = mybir.dt.uint32
    AX = mybir.AxisListType.X
    OP = mybir.AluOpType
    AF = mybir.ActivationFunctionType

    nc = bacc.Bacc("TRN2", target_bir_lowering=False, debug=False)

    # ht[s, q, p, j*SUP + u] = hiddenT[(4q+j)*128 + p, s*SUP + u]
    ht = nc.declare_dram_parameter("ht", [nsup, NQ, P, QJ * SUP], bf16,
                                   isOutput=False)
    wt = nc.declare_dram_parameter("wt", [P, NCH * E], bf16, isOutput=False)
    idb = nc.declare_dram_parameter("idb", [E, E], bf16, isOutput=False)
    oi = nc.declare_dram_parameter("oi", [P, NG, 4], i32, isOutput=True)
    ow = nc.declare_dram_parameter("ow", [P, NG, 4], bf16, isOutput=True)
    ok = nc.declare_dram_parameter("ok", [P, NG], i32, isOutput=True)

    with tile.TileContext(nc) as tc, ExitStack() as ctx:
        const = ctx.enter_context(tc.tile_pool(name="const", bufs=1))
        htp = ctx.enter_context(tc.tile_pool(name="htp", bufs=5))
        mmps = ctx.enter_context(
            tc.tile_pool(name="mmps", bufs=2, space=bass.MemorySpace.PSUM))
        trps = ctx.enter_context(
            tc.tile_pool(name="trps", bufs=2, space=bass.MemorySpace.PSUM))
        work = ctx.enter_context(tc.tile_pool(name="work", bufs=2))
        outp = ctx.enter_context(tc.tile_pool(name="outp", bufs=1))

        wt_s = const.tile([P, NCH * E], bf16)
        nc.scalar.dma_start(wt_s[:], wt.ap())
        identb = const.tile([E, E], bf16)
        nc.scalar.dma_start(identb[:], idb.ap())
        oi_s = outp.tile([P, NG, 4], i32)
        ow_s = outp.tile([P, NG, 4], bf16)
        ok_s = outp.tile([P, NG], i32)

        def emit_mm(s):
            # Column-tiled matmuls: PE columns 0-63 compute the first
            # 512-token tile, columns 64-127 the second, against the same
            # stationary weight chunk. Each PSUM half accumulates its
            # tokens' complete logitsT over all 32 chunks.
            lg_ps = mmps.tile([P, TT], f32, tag="lgps")
            mid_mm = None
            for q in range(NQ):
                htile = htp.tile([P, QJ, SUP], bf16, tag="ht")
                nc.sync.dma_start(htile[:], ht.ap()[s, q])
                for j in range(QJ):
                    c = QJ * q + j
                    for half in range(2):
                        # skip_group_check: the sim's PSUM-group tracker is
                        # not partition-base aware; the halves are disjoint
                        # partition ranges of the bank.
                        mm = nc.tensor.matmul(
                            lg_ps[half * E:(half + 1) * E, :],
                            wt_s[:, bass.ts(c, E)],
                            htile[:, j, bass.ts(half, TT)],
                            start=(c == 0), stop=(c == NCH - 1),
                            tile_position=(0, half * E),
                            skip_group_check=True)
                    if q == NQ // 2 and j == 0:
                        mid_mm = mm
            return lg_ps, mid_mm

        def emit_post(s, lg_ps, anchor):
            # single fp32 -> bf16 rounding (the reference's einsum output)
            sb = work.tile([P, TT], bf16, tag="sb")
            nc.scalar.copy(sb[:], lg_ps[:])
            # move the second half to partition base 0 (HWDGE SBUF copy)
            # so every PE transpose runs in the proven base-0 form
            sb_hi = work.tile([E, TT], bf16, tag="sbhi")
            nc.scalar.dma_start(sb_hi[:], sb[E:P, :])

            for half in range(2):
                t = 2 * s + half
                src = sb[0:E, :] if half == 0 else sb_hi[:]

                # ---- PE transpose to token-major: (128 tok, G, 64 exp)
                tr_ps = trps.tile([P, G, E], bf16, tag="trps")
                for g in range(G):
                    tr = nc.tensor.transpose(
                        tr_ps[:, g, :], src[:, bass.ts(g, P)], identb[:])
                    if anchor is not None:
                        # scheduling-only: keep transposes after the NEXT
                        # supertile's mid-matmul so the PE never head-of-line
                        # stalls on this merge chain
                        add_dep_helper(tr.ins, anchor.ins, sync=False,
                                       reason="tr after next supertile mms")
                lgt = work.tile([P, G, E], bf16, tag="lgt")
                nc.vector.tensor_copy(lgt[:], tr_ps[:])

                # ---- softmax (fp32): y = x - max (single-rounded fp32,
                # bit-identical to ACT's fused bias), one batched exp
                mneg = work.tile([P, G, 1], f32, tag="mneg")
                nc.vector.reduce_max(mneg[:, :, 0], lgt[:], axis=AX, negate=True)
                y = work.tile([P, G, E], f32, tag="y")
                ya, yb = bass.broadcast_tensor_aps(lgt[:], mneg[:])
                nc.vector.tensor_tensor(y[:], ya, yb, OP.add)
                pe_t = work.tile([P, G, E], f32, tag="pe")
                nc.scalar.activation(pe_t[:], y[:], AF.Exp)
                # top-8 first: independent of the entropy chain, packs the
                # DVE FIFO while ACT runs
                mv = work.tile([P, G, 8], f32, tag="mv")
                mi = work.tile([P, G, 8], u32, tag="mi")
                for g in range(G):
                    nc.vector.max(mv[:, g, :], pe_t[:, g, :])
                    nc.vector.max_index(mi[:, g, :], mv[:, g, :], pe_t[:, g, :])
                zs = work.tile([P, G, 1], f32, tag="zs")
                nc.vector.reduce_sum(zs[:, :, 0], pe_t[:], axis=AX)
                rz = work.tile([P, G, 1], f32, tag="rz")
                nc.vector.reciprocal(rz[:, :, 0], zs[:, :, 0])

                # ---- entropy_neg = sum p*ln(p) = rz*sum(pe*y) - ln(Z)
                # (drops the reference's +1e-9 epsilon: |delta| < 1e-7,
                #  three orders below the 3e-5 threshold margins)
                pl = work.tile([P, G, E], f32, tag="pl")
                nc.vector.tensor_tensor(pl[:], pe_t[:], y[:], OP.mult)
                s2 = work.tile([P, G, 1], f32, tag="s2")
                nc.vector.reduce_sum(s2[:, :, 0], pl[:], axis=AX)
                lnz = work.tile([P, G, 1], f32, tag="lnz")
                nc.scalar.activation(lnz[:, :, 0], zs[:, :, 0], AF.Ln)
                e1 = work.tile([P, G, 1], f32, tag="e1")
                nc.vector.tensor_tensor(e1[:], s2[:], rz[:], OP.mult)
                entn = work.tile([P, G], f32, tag="entn")
                nc.vector.tensor_tensor(entn[:], e1[:, :, 0], lnz[:, :, 0],
                                        OP.subtract)

                # ---- adaptive k: entropy<0.3 -> 1, >1.5 -> 4, else 2
                # entn = -entropy: k>=2 iff entn <= -0.3; k==4 iff entn < -1.5
                m2 = work.tile([P, G, 1], f32, tag="m2")
                nc.vector.tensor_scalar(m2[:, :, 0], entn[:], -0.3, None, OP.is_le)
                m4 = work.tile([P, G, 1], f32, tag="m4")
                nc.vector.tensor_scalar(m4[:, :, 0], entn[:], -1.5, None, OP.is_lt)
                kf = work.tile([P, G], f32, tag="kf")
                nc.vector.scalar_tensor_tensor(
                    kf[:], m4[:, :, 0], 2.0, m2[:, :, 0], OP.mult, OP.add)
                nc.vector.tensor_scalar_add(kf[:], kf[:], 1.0)
                nc.vector.tensor_copy(ok_s[:, bass.ts(t, G)], kf[:])

                # ---- active-slot mask (slot0 always, slot1 k>=2, 2/3 k==4)
                act = work.tile([P, G, 4], f32, tag="act")
                nc.vector.memset(act[:, :, 0:1], 1.0)
                nc.vector.tensor_copy(act[:, :, 1:2], m2[:])
                nc.vector.tensor_copy(act[:, :, 2:3], m4[:])
                nc.vector.tensor_copy(act[:, :, 3:4], m4[:])

                # ---- masked renormalized weights
                w4 = work.tile([P, G, 4], f32, tag="w4")
                nc.vector.tensor_tensor(w4[:], mv[:, :, 0:4], act[:], OP.mult)
                ws = work.tile([P, G], f32, tag="ws")
                nc.vector.reduce_sum(ws[:], w4[:], axis=AX)
                rw = work.tile([P, G, 1], f32, tag="rw")
                nc.vector.reciprocal(rw[:, :, 0], ws[:])
                wn = work.tile([P, G, 4], f32, tag="wn")
                wa_bc, wb_bc = bass.broadcast_tensor_aps(w4[:], rw[:])
                nc.vector.tensor_tensor(wn[:], wa_bc, wb_bc, OP.mult)
                nc.vector.tensor_copy(ow_s[:, bass.ts(t, G)], wn[:])

                # ---- indices: (idx+1)*active - 1 (inactive slots -> -1)
                idxf = work.tile([P, G, 4], f32, tag="idxf")
                nc.vector.tensor_copy(idxf[:], mi[:, :, 0:4])
                nc.vector.scalar_tensor_tensor(
                    idxf[:], idxf[:], 1.0, act[:], OP.add, OP.mult)
                nc.vector.tensor_scalar_add(idxf[:], idxf[:], -1.0)
                nc.vector.tensor_copy(oi_s[:, bass.ts(t, G)], idxf[:])

            # per-supertile output stores (only the last sliver is a tail)
            gsl = bass.ts(s, 2 * G)
            nc.sync.dma_start(oi.ap()[:, gsl], oi_s[:, gsl])
            nc.sync.dma_start(ow.ap()[:, gsl], ow_s[:, gsl])
            nc.sync.dma_start(ok.ap()[:, gsl], ok_s[:, gsl])

        # software pipeline: supertile s streams while s-1 post-processes
        prev = None
        for s in range(nsup):
            lg, mid = emit_mm(s)
            if prev is not None:
                emit_post(s - 1, prev, mid)
            prev = lg
        emit_post(nsup - 1, prev, None)

    nc.finalize()
    return nc


def _get_nc():
    if "nc" not in _CACHE:
        _CACHE["nc"] = build_nc()
    return _CACHE["nc"]


def _prep_shards(hidden, weight):
    hidden = np.asarray(hidden)
    weight = np.asarray(weight)
    if hidden.dtype != BF16:
        hidden = hidden.astype(BF16)
    if weight.dtype != BF16:
        weight = weight.astype(BF16)
    # weight (E, H) -> wt[p, c*E + e] = weight[e, c*P + p]
    wt = np.ascontiguousarray(
        weight.reshape(E, NCH, P).transpose(2, 1, 0).reshape(P, NCH * E))
    # hidden (T, H) -> hiddenT (H, T), shard along tokens, pack per-DMA-dense:
    # ht[s, q, p, j*SUP + u] = hiddenT[(QJ*q+j)*P + p, s*SUP + u]
    ht_full = np.ascontiguousarray(hidden.T)
    in_maps = []
    for c in range(N_CORES):
        sh = ht_full[:, c * T_LOC:(c + 1) * T_LOC]          # (H, T_LOC)
        s5 = sh.reshape(NQ, QJ, P, NSUP, SUP)               # (q, j, p, s, u)
        ht_shard = np.ascontiguousarray(
            s5.transpose(3, 0, 2, 1, 4).reshape(NSUP, NQ, P, QJ * SUP))
        in_maps.append({"ht": ht_shard, "wt": wt, "idb": IDB})
    return in_maps


def _assemble(results):
    idx_parts, w_parts, k_parts = [], [], []
    for c in range(N_CORES):
        oi = np.asarray(results[c]["oi"])            # (P, NG, 4) int32
        ow = np.asarray(results[c]["ow"])            # (P, NG, 4) bf16
        ok = np.asarray(results[c]["ok"])            # (P, NG)   int32
        # token = g*128 + p  ->  [g, p, s]
        idx_parts.append(oi.transpose(1, 0, 2).reshape(T_LOC, 4))
        w_parts.append(ow.transpose(1, 0, 2).reshape(T_LOC, 4))
        k_parts.append(ok.transpose(1, 0).reshape(T_LOC))
    indices = np.concatenate(idx_parts, axis=0).astype(np.int32)
    weights = np.concatenate(w_parts, axis=0)
    if weights.dtype != BF16:
        weights = weights.view(BF16) if weights.dtype.itemsize == 2 \
            else weights.astype(BF16)
    k = np.concatenate(k_parts, axis=0).astype(np.int32)
    return indices, weights, k


def kernel(hidden, weight):
    from concourse.bass_utils import run_bass_kernel_spmd

    nc = _get_nc()
    in_maps = _prep_shards(hidden, weight)
    res = run_bass_kernel_spmd(nc, in_maps, core_ids=list(range(N_CORES)))
    return _assemble(res.results)
